# revision 1
# baseline (speedup 1.0000x reference)
"""Cross multi-head attention TRN2 kernel (8-core SPMD, head-sharded).

Strategy (tensor parallel over heads, zero communication):
  - 16 heads / 8 cores -> 2 heads per core. Core c computes output columns
    [128*c, 128*(c+1)) of the [4096, 1024] output; host concatenates.
  - Host pre-transposes q/embed to [E, rows] and casts to bf16 so the
    contraction dim (E) lands on SBUF partitions with no on-chip transposes.
  - Scores are computed transposed (S^T[k, q] = K.Q^T, scale folded into Wq),
    softmax skips the max-subtraction (logits ~ N(0,1), exp is safe in fp32),
    and the denominator is obtained by appending a ones-column to V so the
    attn.V matmul also produces row-sums. ctx\'^T is PE-transposed back to
    [q, d] layout, then normalized per-partition and DMA\'d out.
  - Batch-1 projections are interleaved into batch-0\'s attention so the PE
    fills the gaps of the ACT(exp)-governed attention pipeline.
"""

import numpy as np
import ml_dtypes

import concourse.bass as bass
import concourse.bacc as bacc
import concourse.mybir as mybir
import concourse.tile as tile
from concourse.bass_utils import run_bass_kernel_spmd
from concourse.masks import make_identity

# ---- problem dims (hardcoded; kernel.py must be self-contained) ----
B, S, E = 2, 2048, 1024
NHEAD, HD = 16, 64
NCORES = 8
HPC = NHEAD // NCORES          # heads per core = 2
DPC = HPC * HD                 # projection out-dims per core = 128
ROWS = B * S                   # 4096
P = 128                        # SBUF partitions
NFREE = 512                    # matmul moving free dim (one PSUM bank fp32)
EC = E // P                    # 8 contraction chunks
KC = S // P                    # 16 key chunks per batch
QC = S // NFREE                # 4 query chunks per batch
RC_B = S // NFREE              # 4 projection row-chunks per batch
KGRP = 2                       # k-chunks fused per exp activation
SCALE = 1.0 / np.sqrt(HD)      # 0.125, folded into Wq/bq on host

F32 = mybir.dt.float32
BF16 = mybir.dt.bfloat16
AF = mybir.ActivationFunctionType

_CACHED_NC = {}
LAST_RESULTS = None            # test.py reads exec_time_ns / profile from here


def _build_nc(with_bias: bool) -> bass.Bass:
    nc = bacc.Bacc(
        "TRN2",
        target_bir_lowering=False,
        debug=False,
        num_devices=NCORES,
    )

    qT = nc.declare_dram_parameter("qT", [E, ROWS], BF16, isOutput=False)
    eT = nc.declare_dram_parameter("eT", [E, ROWS], BF16, isOutput=False)
    WqT = nc.declare_dram_parameter("WqT", [E, DPC], BF16, isOutput=False)
    WkT = nc.declare_dram_parameter("WkT", [E, DPC], BF16, isOutput=False)
    WvT = nc.declare_dram_parameter("WvT", [E, DPC], BF16, isOutput=False)
    bqs = nc.declare_dram_parameter("bqs", [DPC], BF16, isOutput=False)
    bkp = nc.declare_dram_parameter("bkp", [DPC], BF16, isOutput=False)
    bvp = nc.declare_dram_parameter("bvp", [DPC], BF16, isOutput=False)
    out = nc.declare_dram_parameter("out", [ROWS, DPC], F32, isOutput=True)

    with tile.TileContext(nc) as tc:
        with (
            tc.tile_pool(name="consts", bufs=1) as consts,
            tc.tile_pool(name="wpool", bufs=1) as wpool,
            tc.tile_pool(name="resid", bufs=1) as resid,
            tc.tile_pool(name="src", bufs=3) as srcp,
            tc.tile_pool(name="probs", bufs=3) as prp,
            tc.tile_pool(name="misc", bufs=3) as misc,
            tc.tile_pool(name="otp", bufs=10) as otp,
            tc.tile_pool(name="psmall", bufs=2, space="PSUM") as psmall,
            tc.tile_pool(name="psq", bufs=2, space="PSUM") as psq,
            tc.tile_pool(name="pctx", bufs=2, space="PSUM") as pctx,
        ):
            # ---------- constants & weights ----------
            wq_sb = wpool.tile([P, EC, DPC], BF16)
            nc.sync.dma_start(wq_sb, WqT.ap().rearrange("(c p) d -> p c d", p=P))
            wk_sb = wpool.tile([P, EC, DPC], BF16)
            nc.sync.dma_start(wk_sb, WkT.ap().rearrange("(c p) d -> p c d", p=P))
            wv_sb = wpool.tile([P, EC, DPC], BF16)
            nc.sync.dma_start(wv_sb, WvT.ap().rearrange("(c p) d -> p c d", p=P))

            ident = consts.tile([P, P], F32)
            make_identity(nc, ident)
            ones_row = consts.tile([1, NFREE], BF16)
            nc.vector.memset(ones_row, 1.0)

            bq_sb = wpool.tile([1, DPC], BF16)
            nc.gpsimd.dma_start(bq_sb, bqs.ap()[None, :])
            bk_sb = wpool.tile([1, DPC], BF16)
            nc.gpsimd.dma_start(bk_sb, bkp.ap()[None, :])
            bv_sb = wpool.tile([1, DPC], BF16)
            nc.gpsimd.dma_start(bv_sb, bvp.ap()[None, :])

            # ---------- residents (per batch) ----------
            qt_sb = []
            kt_sb = []
            v_sb = []
            for b in range(B):
                qt = resid.tile([P, S], BF16, name=f"qt{b}")
                kt = resid.tile([P, S], BF16, name=f"kt{b}")
                vv = resid.tile([P, KC, HPC, HD + 1], BF16, name=f"v{b}")
                nc.vector.memset(vv[:, :, :, HD : HD + 1], 1.0)
                qt_sb.append(qt)
                kt_sb.append(kt)
                v_sb.append(vv)

            HEC = EC // 2

            def proj_pieces(b, r):
                """Projections for 512 rows of batch b, as 5 schedulable
                pieces: (dma), (Qproj), (Kproj), (V 0-1), (V 2-3)."""
                row0 = b * S + r * NFREE
                col0 = r * NFREE
                halves = {}

                def do_dma():
                    for key, dram in (("q", qT), ("e", eT)):
                        tiles = []
                        for hh in range(2):
                            tl = srcp.tile([P, HEC, NFREE], BF16, tag=f"{key}src")
                            nc.sync.dma_start(
                                tl,
                                dram.ap()[
                                    hh * HEC * P : (hh + 1) * HEC * P,
                                    row0 : row0 + NFREE,
                                ].rearrange("(c p) n -> p c n", p=P),
                            )
                            tiles.append(tl)
                        halves[key] = tiles

                def sl(key, c):
                    return halves[key][c // HEC][:, c % HEC]

                def qk_proj(w_t, b_t, dst, key):
                    pp = psmall.tile([P, NFREE], F32, tag="ps")
                    for c in range(EC):
                        nc.tensor.matmul(
                            pp,
                            lhsT=w_t[:, c],
                            rhs=sl(key, c),
                            start=(c == 0),
                            stop=(not with_bias and c == EC - 1),
                        )
                    if with_bias:
                        # bias: rank-1 update b[d] (x) ones(rows)
                        nc.tensor.matmul(
                            pp, lhsT=b_t, rhs=ones_row, start=False, stop=True
                        )
                    nc.vector.tensor_copy(dst[:, col0 : col0 + NFREE], pp)

                def v_proj(sub):
                    kc = r * (NFREE // P) + sub
                    pv = psmall.tile([P, DPC], F32, tag="ps")
                    for c in range(EC):
                        nc.tensor.matmul(
                            pv,
                            lhsT=sl("e", c)[:, sub * P : (sub + 1) * P],
                            rhs=wv_sb[:, c],
                            start=(c == 0),
                            stop=(not with_bias and c == EC - 1),
                        )
                    if with_bias:
                        # bias via K=1 outer product (bv bcast to all rows)
                        nc.tensor.matmul(
                            pv,
                            lhsT=ones_row[:, :P],
                            rhs=bv_sb,
                            start=False,
                            stop=True,
                        )
                    for h in range(HPC):
                        nc.vector.tensor_copy(
                            v_sb[b][:, kc, h, 0:HD], pv[:, h * HD : (h + 1) * HD]
                        )

                return [
                    do_dma,
                    lambda: qk_proj(wq_sb, bq_sb, qt_sb[b], "q"),
                    lambda: qk_proj(wk_sb, bk_sb, kt_sb[b], "e"),
                    lambda: (v_proj(0), v_proj(1)),
                    lambda: (v_proj(2), v_proj(3)),
                ]

            def proj_chunk(b, r):
                for piece in proj_pieces(b, r):
                    piece()

            def attn_iter(b, h, qc, ot_tiles):
                """Attention for one (batch, head, 512-query chunk)."""
                d0 = h * HD
                col0 = qc * NFREE
                ctx_ps = pctx.tile([HD + 1, NFREE], F32, tag="ctx")
                for g in range(KC // KGRP):
                    sp = psq.tile([P, KGRP * NFREE], F32, tag="sps")
                    for j in range(KGRP):
                        kc = g * KGRP + j
                        nc.tensor.matmul(
                            sp[:, j * NFREE : (j + 1) * NFREE],
                            lhsT=kt_sb[b][d0 : d0 + HD, kc * P : (kc + 1) * P],
                            rhs=qt_sb[b][d0 : d0 + HD, col0 : col0 + NFREE],
                            start=True,
                            stop=True,
                        )
                    pr = prp.tile([P, KGRP * NFREE], BF16, tag="pr")
                    nc.scalar.activation(pr, sp, AF.Exp)
                    for j in range(KGRP):
                        kc = g * KGRP + j
                        nc.tensor.matmul(
                            ctx_ps,
                            lhsT=v_sb[b][:, kc, h, :],
                            rhs=pr[:, j * NFREE : (j + 1) * NFREE],
                            start=(kc == 0),
                            stop=(kc == KC - 1),
                        )
                # ctx\'^T [65, 512]: transpose 128-col chunks, normalize
                ctxT = misc.tile([HD + 1, NFREE], F32, tag="ctxT")
                nc.vector.tensor_copy(ctxT, ctx_ps)
                for t in range(NFREE // P):
                    tp = psmall.tile([P, HD + 1], F32, tag="ps")
                    nc.tensor.transpose(
                        tp,
                        ctxT[:, t * P : (t + 1) * P],
                        ident[: HD + 1, : HD + 1],
                    )
                    rcp = misc.tile([P, 1], F32, tag="rcp")
                    nc.vector.reciprocal(rcp, tp[:, HD : HD + 1])
                    nc.vector.tensor_mul(
                        ot_tiles[t][:, d0 : d0 + HD],
                        tp[:, 0:HD],
                        rcp.broadcast_to([P, HD]),
                    )
                    if h == HPC - 1:
                        row0 = b * S + qc * NFREE + t * P
                        nc.sync.dma_start(
                            out.ap()[row0 : row0 + P, :], ot_tiles[t]
                        )

            # program order: proj(b0); attn(b0) with proj(b1) interleaved
            # (PE fills ACT-governed gaps); attn(b1).
            def attn_qc(b, qc, fillers):
                ot_tiles = [
                    otp.tile([P, DPC], F32, tag="ot", name=f"ot{b}_{qc}_{t}")
                    for t in range(NFREE // P)
                ]
                for h in range(HPC):
                    attn_iter(b, h, qc, ot_tiles)
                    if fillers:
                        fillers.pop(0)()

            for r in range(RC_B):
                proj_chunk(0, r)
            # b1 projections emitted piecewise between b0 attention iters so
            # the static per-engine order keeps ACT fed while PE does proj.
            fillers = []
            for r in range(RC_B):
                fillers.extend(proj_pieces(1, r))
            # pieces reference live src tiles; DMAs for chunk r are emitted at
            # proj_pieces() call time above -- keep srcp deep enough.
            for qc in range(QC):
                attn_qc(0, qc, fillers)
            while fillers:
                fillers.pop(0)()
            for qc in range(QC):
                attn_qc(1, qc, [])

    nc.finalize()
    return nc


def _get_nc(with_bias: bool = True) -> bass.Bass:
    if with_bias not in _CACHED_NC:
        _CACHED_NC[with_bias] = _build_nc(with_bias)
    return _CACHED_NC[with_bias]


def kernel(embed, q, Wk, bk, Wq, bq, Wv, bv, trace=False):
    global LAST_RESULTS
    bf = ml_dtypes.bfloat16
    embed = np.asarray(embed, dtype=np.float32)
    q = np.asarray(q, dtype=np.float32)
    Wk = np.asarray(Wk, dtype=np.float32)
    Wq = np.asarray(Wq, dtype=np.float32)
    Wv = np.asarray(Wv, dtype=np.float32)
    bk = np.asarray(bk, dtype=np.float32)
    bq = np.asarray(bq, dtype=np.float32)
    bv = np.asarray(bv, dtype=np.float32)

    qT = np.ascontiguousarray(q.reshape(ROWS, E).T).astype(bf)
    eT = np.ascontiguousarray(embed.reshape(ROWS, E).T).astype(bf)

    in_maps = []
    for c in range(NCORES):
        sl = slice(c * DPC, (c + 1) * DPC)
        in_maps.append(
            {
                "qT": qT,
                "eT": eT,
                # scores scale folded into Wq/bq (exact: *2^-3)
                "WqT": np.ascontiguousarray((Wq[sl] * SCALE).T).astype(bf),
                "WkT": np.ascontiguousarray(Wk[sl].T).astype(bf),
                "WvT": np.ascontiguousarray(Wv[sl].T).astype(bf),
                "bqs": (bq[sl] * SCALE).astype(bf),
                "bkp": bk[sl].astype(bf),
                "bvp": bv[sl].astype(bf),
            }
        )

    with_bias = bool(bq.any() or bk.any() or bv.any())
    nc = _get_nc(with_bias)
    res = run_bass_kernel_spmd(nc, in_maps, list(range(NCORES)), trace=trace)
    LAST_RESULTS = res

    full = np.empty((ROWS, E), dtype=np.float32)
    for c in range(NCORES):
        full[:, c * DPC : (c + 1) * DPC] = res.results[c]["out"]
    return full.reshape(B, S, E)



# revision 18
# speedup vs baseline: 1.0440x; 1.0440x over previous
"""Cross multi-head attention TRN2 kernel (8-core SPMD, head-sharded).

Strategy (tensor parallel over heads, zero communication):
  - 16 heads / 8 cores -> 2 heads per core. Core c computes output columns
    [128*c, 128*(c+1)) of the [4096, 1024] output; host concatenates.
  - Host pre-transposes q/embed to [E, rows] and casts to fp16 (10-bit
    mantissa: ~8x less error than bf16 at identical PE speed) so the
    contraction dim (E) lands on SBUF partitions with no on-chip transposes.
  - Scores are computed transposed (S^T[k, q] = K.Q^T, scale folded into Wq),
    softmax skips the max-subtraction (logits ~ N(0,1), exp is safe in fp32),
    and the denominator is obtained by appending a ones-column to V so the
    attn.V matmul also produces row-sums. ctx'^T is PE-transposed back to
    [q, d] layout, then normalized per-partition and DMA'd out.
  - Schedule: the attention inner loop is ACT(exp)-bound (~1.05us per
    [128,1024] exp vs ~0.85us of PE per group). All projection work beyond a
    minimal prologue (K(b0) fully + Q(b0,r0..1) + V(b0,r0)) is split into
    ~850ns half-pieces and woven into the attention groups' ACT slack, so PE
    never idles (idle PE drops to the 1.2GHz p-state and re-ramps slowly).
    Per-iteration drains (ctx transpose + normalize + store) are deferred
    into the next iteration's slots for the same reason.
"""

import numpy as np

import concourse.bass as bass
import concourse.bacc as bacc
import concourse.mybir as mybir
import concourse.tile as tile
from concourse.bass_utils import run_bass_kernel_spmd
from concourse.masks import make_identity

# ---- problem dims (hardcoded; kernel.py must be self-contained) ----
B, S, E = 2, 2048, 1024
NHEAD, HD = 16, 64
NCORES = 8
HPC = NHEAD // NCORES          # heads per core = 2
DPC = HPC * HD                 # projection out-dims per core = 128
ROWS = B * S                   # 4096
P = 128                        # SBUF partitions
NFREE = 512                    # matmul moving free dim (one PSUM bank fp32)
EC = E // P                    # 8 contraction chunks
KC = S // P                    # 16 key chunks per batch
QC = S // NFREE                # 4 query chunks per batch
RC_B = S // NFREE              # 4 projection row-chunks per batch
KGRP = 2                       # k-chunks fused per exp activation
GRP = KC // KGRP               # groups (slots) per attention iter = 8
SCALE = 1.0 / np.sqrt(HD)      # 0.125, folded into Wq/bq on host (exact 2^-3)

F32 = mybir.dt.float32
F16 = mybir.dt.float16
AF = mybir.ActivationFunctionType

_CACHED_NC = {}
LAST_RESULTS = None            # test.py reads exec_time_ns / profile from here


def _build_nc(with_bias: bool) -> bass.Bass:
    nc = bacc.Bacc(
        "TRN2",
        target_bir_lowering=False,
        debug=False,
        num_devices=NCORES,
    )

    # Host pre-blocks inputs/weights so every DMA is 128 descriptors of
    # contiguous >=2KB per partition (SWDGE descriptor generation is ~15ns
    # per descriptor; fine-grained rearranges cost 15us+ per load).
    qT = nc.declare_dram_parameter("qT", [B * RC_B * P, EC * NFREE], F16, isOutput=False)
    eT = nc.declare_dram_parameter("eT", [B * RC_B * P, EC * NFREE], F16, isOutput=False)
    WqT = nc.declare_dram_parameter("WqT", [P, EC * DPC], F16, isOutput=False)
    WkT = nc.declare_dram_parameter("WkT", [P, EC * DPC], F16, isOutput=False)
    WvT = nc.declare_dram_parameter("WvT", [P, EC * DPC], F16, isOutput=False)
    bqs = nc.declare_dram_parameter("bqs", [DPC], F16, isOutput=False)
    bkp = nc.declare_dram_parameter("bkp", [DPC], F16, isOutput=False)
    bvp = nc.declare_dram_parameter("bvp", [DPC], F16, isOutput=False)
    out = nc.declare_dram_parameter("out", [ROWS, DPC], F32, isOutput=True)

    with tile.TileContext(nc) as tc:
        with (
            tc.tile_pool(name="consts", bufs=1) as consts,
            tc.tile_pool(name="wpool", bufs=1) as wpool,
            tc.tile_pool(name="resid", bufs=1) as resid,
            tc.tile_pool(name="src", bufs=8) as srcp,
            tc.tile_pool(name="probs", bufs=3) as prp,
            tc.tile_pool(name="ctp", bufs=2) as ctp,
            tc.tile_pool(name="misc", bufs=6) as misc,
            tc.tile_pool(name="otp", bufs=12) as otp,
            tc.tile_pool(name="psq", bufs=2, space="PSUM") as psq,
            tc.tile_pool(name="pctx", bufs=1, space="PSUM") as pctx,
            tc.tile_pool(name="pproj", bufs=1, space="PSUM") as pproj,
            tc.tile_pool(name="ptr", bufs=2, space="PSUM") as ptr,
        ):
            # ---------- constants & weights ----------
            # K first (prologue), then V, then Q — on the gpsimd ring ahead
            # of the q-source loads; scalar ring carries only the tiny biases.
            wk_sb = wpool.tile([P, EC, DPC], F16)
            nc.gpsimd.dma_start(wk_sb, WkT.ap().rearrange("p (c d) -> p c d", d=DPC))
            wv_sb = wpool.tile([P, EC, DPC], F16)
            nc.gpsimd.dma_start(wv_sb, WvT.ap().rearrange("p (c d) -> p c d", d=DPC))
            wq_sb = wpool.tile([P, EC, DPC], F16)
            nc.gpsimd.dma_start(wq_sb, WqT.ap().rearrange("p (c d) -> p c d", d=DPC))

            ident = consts.tile([P, P], F16)
            make_identity(nc, ident)
            ones_row = consts.tile([1, NFREE], F16)
            nc.vector.memset(ones_row, 1.0)

            bq_sb = wpool.tile([1, DPC], F16)
            nc.scalar.dma_start(bq_sb, bqs.ap()[None, :])
            bk_sb = wpool.tile([1, DPC], F16)
            nc.scalar.dma_start(bk_sb, bkp.ap()[None, :])
            bv_sb = wpool.tile([1, DPC], F16)
            nc.scalar.dma_start(bv_sb, bvp.ap()[None, :])

            # ---------- residents (per batch) ----------
            qt_sb, kt_sb, v_sb = [], [], []
            for b in range(B):
                qt_sb.append(resid.tile([P, S], F16, name=f"qt{b}"))
                kt_sb.append(resid.tile([P, S], F16, name=f"kt{b}"))
                vv = resid.tile([P, KC, HPC, HD + 1], F16, name=f"v{b}")
                nc.vector.memset(vv[:, :, :, HD : HD + 1], 1.0)
                v_sb.append(vv)

            # ---------- input DMAs (e on sync ring, q on gpsimd ring) ----
            # One tile per (tensor, b, r): [128, 8 e-chunks, 512 rows], host
            # pre-blocked so each load is 128 x 8KB contiguous. e/q tiles are
            # interleaved in allocation order so the 8 pool slots cover all
            # of b0 at once; b1 loads start as b0 slots free up.
            src = {}
            for b in range(B):
                for r in range(RC_B):
                    blk = (b * RC_B + r) * P
                    t = srcp.tile([P, EC, NFREE], F16, tag="src", name=f"e{b}{r}")
                    nc.sync.dma_start(
                        t,
                        eT.ap()[blk : blk + P, :].rearrange(
                            "p (c n) -> p c n", n=NFREE
                        ),
                    )
                    src[("e", b, r)] = t
                    t = srcp.tile([P, EC, NFREE], F16, tag="src", name=f"q{b}{r}")
                    nc.gpsimd.dma_start(
                        t,
                        qT.ap()[blk : blk + P, :].rearrange(
                            "p (c n) -> p c n", n=NFREE
                        ),
                    )
                    src[("q", b, r)] = t

            # ---------- projection half-pieces (~850ns of PE each) ------
            HEC = EC // 2
            qk_acc = {}

            def qk_half(key, b, r, half, w_t, b_t, dst):
                """Half of a Q/K projection chunk: 4 contraction matmuls;
                second half adds bias and copies PSUM -> resident tile."""
                xs = src[(key, b, r)]
                if half == 0:
                    qk_acc[(key, b, r)] = pproj.tile(
                        [P, NFREE], F32, tag="pj", name=f"pj_{key}{b}{r}"
                    )
                pp = qk_acc[(key, b, r)]
                for c in range(half * HEC, half * HEC + HEC):
                    nc.tensor.matmul(
                        pp,
                        lhsT=w_t[:, c],
                        rhs=xs[:, c],
                        start=(c == 0),
                        stop=(not with_bias and c == EC - 1),
                    )
                if half == 1:
                    if with_bias:
                        nc.tensor.matmul(
                            pp, lhsT=b_t, rhs=ones_row, start=False, stop=True
                        )
                    col0 = r * NFREE
                    nc.vector.tensor_copy(dst[:, col0 : col0 + NFREE], pp)

            def v_half(b, r, pair):
                """Two 128-row V sub-chunks (kc = 4r+2*pair, +1)."""
                xs = src[("e", b, r)]
                for sub in (2 * pair, 2 * pair + 1):
                    kc = r * (NFREE // P) + sub
                    # ptr pool (not pproj): pproj buf may hold an open Q/K
                    # accumulation across interleaved filler pieces.
                    pv = ptr.tile([P, HPC, HD], F32, tag="tp")
                    for c in range(EC):
                        nc.tensor.matmul(
                            pv,
                            lhsT=xs[:, c, sub * P : (sub + 1) * P],
                            rhs=wv_sb[:, c],
                            start=(c == 0),
                            stop=(not with_bias and c == EC - 1),
                        )
                    if with_bias:
                        nc.tensor.matmul(
                            pv,
                            lhsT=ones_row[:, :P],
                            rhs=bv_sb,
                            start=False,
                            stop=True,
                        )
                    nc.vector.tensor_copy(v_sb[b][:, kc, :, 0:HD], pv)

            def piece(kind, b, r, i):
                if kind == "K":
                    return lambda: qk_half("e", b, r, i, wk_sb, bk_sb, kt_sb[b])
                if kind == "Q":
                    return lambda: qk_half("q", b, r, i, wq_sb, bq_sb, qt_sb[b])
                return lambda: v_half(b, r, i)

            # ---------- filler queue (proj pieces woven into attention) --
            # Slots are attention groups, numbered globally 0..127.
            SPI = GRP                       # slots per iter
            ipb = QC * HPC                  # iters per batch = 8

            def slot_of(b, it, g):
                return (b * ipb + it) * SPI + g

            fillers = []  # (need_by_slot, piece_fn) sorted by emission
            for b in range(B):
                base = b * ipb * SPI
                nxt = (b + 1) * ipb * SPI
                # V(b, r1..r3): piece (r, pair) feeds PV group 2r+pair of this
                # batch's first iter (slot base+2r+pair); pop one slot earlier.
                for r in range(1, RC_B):
                    for pr_ in range(2):
                        fillers.append(
                            (base + 2 * r + pr_ - 1, piece("V", b, r, pr_))
                        )
                # Q(b, r): feeds iters with qc >= r (first: it = r*HPC);
                # r0 comes from the prologue (b0) / next-batch fillers (b1).
                for r in (1, 2, 3):
                    need = base + (r * HPC) * SPI - 1
                    fillers.append((need, piece("Q", b, r, 0)))
                    fillers.append((need, piece("Q", b, r, 1)))
                # K(b, r): QK is issued 2 groups ahead, so group 2r's QK
                # (reading kt cols of chunk r) is emitted at slot 2r-2.
                if b == 0:
                    for r in (2, 3):
                        fillers.append((2 * r - 3, piece("K", b, r, 0)))
                        fillers.append((2 * r - 3, piece("K", b, r, 1)))
                if b + 1 < B:
                    # next batch prologue-equivalent: K, V r0, Q r0
                    for r in range(RC_B):
                        need = nxt - 1 if r < 2 else nxt - 1 + 2 * (r - 1)
                        fillers.append((need, piece("K", b + 1, r, 0)))
                        fillers.append((need, piece("K", b + 1, r, 1)))
                    fillers.append((nxt - 2, piece("V", b + 1, 0, 0)))
                    fillers.append((nxt - 1, piece("V", b + 1, 0, 1)))
                    fillers.append((nxt - 1, piece("Q", b + 1, 0, 0)))
                    fillers.append((nxt - 1, piece("Q", b + 1, 0, 1)))
            fillers.sort(key=lambda x: x[0])

            AHEAD = 40  # eligibility window (slots) for early emission

            def pop_fillers(slot, force_only=False):
                # forced (dependency) pops: unlimited; early pops: <=1/slot
                while fillers and fillers[0][0] <= slot:
                    fillers.pop(0)[1]()
                if not force_only and fillers and fillers[0][0] <= slot + AHEAD:
                    fillers.pop(0)[1]()

            # ---------- attention ----------
            drains = []   # deferred per-iter drain pieces (run next iter)
            ot_cur = {}   # (b, qc) -> 4 output tiles [128 rows, 128 dpc]

            def emit_qk(b, h, qc, g):
                d0 = h * HD
                col0 = qc * NFREE
                sp = psq.tile([P, KGRP * NFREE], F32, tag="sps")
                for j in range(KGRP):
                    kc = g * KGRP + j
                    nc.tensor.matmul(
                        sp[:, j * NFREE : (j + 1) * NFREE],
                        lhsT=kt_sb[b][d0 : d0 + HD, kc * P : (kc + 1) * P],
                        rhs=qt_sb[b][d0 : d0 + HD, col0 : col0 + NFREE],
                        start=True,
                        stop=True,
                    )
                pr = prp.tile([P, KGRP * NFREE], F16, tag="pr")
                nc.scalar.activation(pr, sp, AF.Exp)
                return pr

            def make_drain(b, h, qc, ctx_ps, ot_tiles):
                """Drain pieces for one finished iter: ctx copy (DVE), then
                per-128-row-block transpose + normalize (+ store at h=1)."""
                ctxT = ctp.tile([HD + 1, NFREE], F16, tag="ctxT")
                d0 = h * HD

                def dcopy():
                    nc.vector.tensor_copy(ctxT, ctx_ps)

                def dblk(t):
                    def run():
                        tp = ptr.tile([P, HD + 1], F16, tag="tp")
                        nc.tensor.transpose(
                            tp,
                            ctxT[:, t * P : (t + 1) * P],
                            ident[: HD + 1, : HD + 1],
                        )
                        rcp = misc.tile([P, 1], F32, tag="rcp")
                        nc.vector.reciprocal(rcp, tp[:, HD : HD + 1])
                        nc.vector.tensor_mul(
                            ot_tiles[t][:, d0 : d0 + HD],
                            tp[:, 0:HD],
                            rcp.broadcast_to([P, HD]),
                        )
                        if h == HPC - 1:
                            row0 = b * S + qc * NFREE + t * P
                            nc.sync.dma_start(
                                out.ap()[row0 : row0 + P, :], ot_tiles[t]
                            )

                    return run

                return [dcopy] + [dblk(t) for t in range(NFREE // P)]

            def attn_iter(b, it):
                qc, h = it // HPC, it % HPC
                if h == 0:
                    ot_cur[(b, qc)] = [
                        otp.tile([P, DPC], F32, tag="ot", name=f"ot{b}_{qc}_{t}")
                        for t in range(NFREE // P)
                    ]
                ot_tiles = ot_cur[(b, qc)]
                ctx_ps = pctx.tile([HD + 1, NFREE], F32, tag="ctx")
                prs = {}
                prs[0] = emit_qk(b, h, qc, 0)
                prs[1] = emit_qk(b, h, qc, 1)
                for g in range(GRP):
                    slot = slot_of(b, it, g)
                    # PV of group g
                    for j in range(KGRP):
                        kc = g * KGRP + j
                        nc.tensor.matmul(
                            ctx_ps,
                            lhsT=v_sb[b][:, kc, h, :],
                            rhs=prs[g][:, j * NFREE : (j + 1) * NFREE],
                            start=(kc == 0),
                            stop=(kc == KC - 1),
                        )
                    if g + 2 < GRP:
                        prs[g + 2] = emit_qk(b, h, qc, g + 2)
                    if drains:
                        drains.pop(0)()
                    pop_fillers(slot, force_only=(slot < SPI))
                drains.extend(make_drain(b, h, qc, ctx_ps, ot_tiles))

            # ---------- prologue: minimal proj before attention ----------
            # K r0..r1 (feeds QK groups 0..3; r2/r3 are slot-0/2 fillers),
            # V r0 (feeds PV groups 0..1), Q r0 (feeds qc0 iters).
            for r in (0, 1):
                qk_half("e", 0, r, 0, wk_sb, bk_sb, kt_sb[0])
                qk_half("e", 0, r, 1, wk_sb, bk_sb, kt_sb[0])
            v_half(0, 0, 0)
            v_half(0, 0, 1)
            qk_half("q", 0, 0, 0, wq_sb, bq_sb, qt_sb[0])
            qk_half("q", 0, 0, 1, wq_sb, bq_sb, qt_sb[0])

            # ---------- main loop ----------
            for b in range(B):
                for it in range(ipb):
                    attn_iter(b, it)
            while drains:
                drains.pop(0)()
            while fillers:
                fillers.pop(0)[1]()

    nc.finalize()
    return nc


def _block_src(x):
    """[B, S, E] fp32 -> [B*RC*P, EC*NFREE] fp16 with tile (b, r) holding
    [p, c*NFREE + n] = x[b, r*NFREE + n, c*P + p] (contiguous per partition)."""
    b5 = x.reshape(B, RC_B, NFREE, EC, P).transpose(0, 1, 4, 3, 2)
    return np.ascontiguousarray(b5).reshape(B * RC_B * P, EC * NFREE).astype(np.float16)


def _block_w(w):
    """[DPC, E] fp32 -> [P, EC*DPC] fp16: [p, c*DPC + d] = w[d, c*P + p]."""
    b3 = w.T.reshape(EC, P, DPC).transpose(1, 0, 2)
    return np.ascontiguousarray(b3).reshape(P, EC * DPC).astype(np.float16)


def _get_nc(with_bias: bool = True) -> bass.Bass:
    if with_bias not in _CACHED_NC:
        _CACHED_NC[with_bias] = _build_nc(with_bias)
    return _CACHED_NC[with_bias]


def kernel(embed, q, Wk, bk, Wq, bq, Wv, bv, trace=False):
    global LAST_RESULTS
    embed = np.asarray(embed, dtype=np.float32)
    q = np.asarray(q, dtype=np.float32)
    Wk = np.asarray(Wk, dtype=np.float32)
    Wq = np.asarray(Wq, dtype=np.float32)
    Wv = np.asarray(Wv, dtype=np.float32)
    bk = np.asarray(bk, dtype=np.float32)
    bq = np.asarray(bq, dtype=np.float32)
    bv = np.asarray(bv, dtype=np.float32)

    qT = _block_src(q)
    eT = _block_src(embed)

    in_maps = []
    for c in range(NCORES):
        sl = slice(c * DPC, (c + 1) * DPC)
        in_maps.append(
            {
                "qT": qT,
                "eT": eT,
                # scores scale folded into Wq/bq (exact: *2^-3)
                "WqT": _block_w(Wq[sl] * SCALE),
                "WkT": _block_w(Wk[sl]),
                "WvT": _block_w(Wv[sl]),
                "bqs": (bq[sl] * SCALE).astype(np.float16),
                "bkp": bk[sl].astype(np.float16),
                "bvp": bv[sl].astype(np.float16),
            }
        )

    with_bias = bool(bq.any() or bk.any() or bv.any())
    nc = _get_nc(with_bias)
    res = run_bass_kernel_spmd(nc, in_maps, list(range(NCORES)), trace=trace)
    LAST_RESULTS = res

    full = np.empty((ROWS, E), dtype=np.float32)
    for c in range(NCORES):
        full[:, c * DPC : (c + 1) * DPC] = res.results[c]["out"]
    return full.reshape(B, S, E)


# revision 19
# speedup vs baseline: 1.0685x; 1.0235x over previous
"""Cross multi-head attention TRN2 kernel (8-core SPMD, head-sharded).

Strategy (tensor parallel over heads, zero communication):
  - 16 heads / 8 cores -> 2 heads per core. Core c computes output columns
    [128*c, 128*(c+1)) of the [4096, 1024] output; host concatenates.
  - Host pre-transposes q/embed to [E, rows] and casts to fp16 (10-bit
    mantissa: ~8x less error than bf16 at identical PE speed) so the
    contraction dim (E) lands on SBUF partitions with no on-chip transposes.
  - Scores are computed transposed (S^T[k, q] = K.Q^T, scale folded into Wq),
    softmax skips the max-subtraction (logits ~ N(0,1), exp is safe in fp32),
    and the denominator is obtained by appending a ones-column to V so the
    attn.V matmul also produces row-sums. ctx'^T is PE-transposed back to
    [q, d] layout, then normalized per-partition and DMA'd out.
  - Schedule: the attention inner loop is ACT(exp)-bound (~1.05us per
    [128,1024] exp vs ~0.85us of PE per group). All projection work beyond a
    minimal prologue (K(b0) fully + Q(b0,r0..1) + V(b0,r0)) is split into
    ~850ns half-pieces and woven into the attention groups' ACT slack, so PE
    never idles (idle PE drops to the 1.2GHz p-state and re-ramps slowly).
    Per-iteration drains (ctx transpose + normalize + store) are deferred
    into the next iteration's slots for the same reason.
"""

import numpy as np

import concourse.bass as bass
import concourse.bacc as bacc
import concourse.mybir as mybir
import concourse.tile as tile
from concourse.bass_utils import run_bass_kernel_spmd
from concourse.masks import make_identity

# ---- problem dims (hardcoded; kernel.py must be self-contained) ----
B, S, E = 2, 2048, 1024
NHEAD, HD = 16, 64
NCORES = 8
HPC = NHEAD // NCORES          # heads per core = 2
DPC = HPC * HD                 # projection out-dims per core = 128
ROWS = B * S                   # 4096
P = 128                        # SBUF partitions
NFREE = 512                    # matmul moving free dim (one PSUM bank fp32)
EC = E // P                    # 8 contraction chunks
KC = S // P                    # 16 key chunks per batch
QC = S // NFREE                # 4 query chunks per batch
RC_B = S // NFREE              # 4 projection row-chunks per batch
KGRP = 2                       # k-chunks fused per exp activation
GRP = KC // KGRP               # groups (slots) per attention iter = 8
SCALE = 1.0 / np.sqrt(HD)      # 0.125, folded into Wq/bq on host (exact 2^-3)

F32 = mybir.dt.float32
F16 = mybir.dt.float16
AF = mybir.ActivationFunctionType

_CACHED_NC = {}
LAST_RESULTS = None            # test.py reads exec_time_ns / profile from here


def _build_nc(with_bias: bool) -> bass.Bass:
    nc = bacc.Bacc(
        "TRN2",
        target_bir_lowering=False,
        debug=False,
        num_devices=NCORES,
    )

    # Host pre-blocks inputs/weights so every DMA is 128 descriptors of
    # contiguous >=2KB per partition (SWDGE descriptor generation is ~15ns
    # per descriptor; fine-grained rearranges cost 15us+ per load).
    qT = nc.declare_dram_parameter("qT", [B * RC_B * P, EC * NFREE], F16, isOutput=False)
    eT = nc.declare_dram_parameter("eT", [B * RC_B * P, EC * NFREE], F16, isOutput=False)
    WqT = nc.declare_dram_parameter("WqT", [P, EC * DPC], F16, isOutput=False)
    WkT = nc.declare_dram_parameter("WkT", [P, EC * DPC], F16, isOutput=False)
    WvT = nc.declare_dram_parameter("WvT", [P, EC * DPC], F16, isOutput=False)
    bqs = nc.declare_dram_parameter("bqs", [DPC], F16, isOutput=False)
    bkp = nc.declare_dram_parameter("bkp", [DPC], F16, isOutput=False)
    bvp = nc.declare_dram_parameter("bvp", [DPC], F16, isOutput=False)
    out = nc.declare_dram_parameter("out", [ROWS, DPC], F32, isOutput=True)

    with tile.TileContext(nc) as tc:
        with (
            tc.tile_pool(name="consts", bufs=1) as consts,
            tc.tile_pool(name="wpool", bufs=1) as wpool,
            tc.tile_pool(name="resid", bufs=1) as resid,
            tc.tile_pool(name="src", bufs=16) as srcp,
            tc.tile_pool(name="probs", bufs=3) as prp,
            tc.tile_pool(name="ctp", bufs=2) as ctp,
            tc.tile_pool(name="misc", bufs=6) as misc,
            tc.tile_pool(name="otp", bufs=20) as otp,
            tc.tile_pool(name="psq", bufs=2, space="PSUM") as psq,
            tc.tile_pool(name="pctx", bufs=1, space="PSUM") as pctx,
            tc.tile_pool(name="pproj", bufs=1, space="PSUM") as pproj,
            tc.tile_pool(name="ptr", bufs=2, space="PSUM") as ptr,
        ):
            # ---------- constants & weights ----------
            # K first (prologue), then V, then Q — on the gpsimd ring ahead
            # of the q-source loads; scalar ring carries only the tiny biases.
            wk_sb = wpool.tile([P, EC, DPC], F16)
            nc.gpsimd.dma_start(wk_sb, WkT.ap().rearrange("p (c d) -> p c d", d=DPC))
            wv_sb = wpool.tile([P, EC, DPC], F16)
            nc.gpsimd.dma_start(wv_sb, WvT.ap().rearrange("p (c d) -> p c d", d=DPC))
            wq_sb = wpool.tile([P, EC, DPC], F16)
            nc.gpsimd.dma_start(wq_sb, WqT.ap().rearrange("p (c d) -> p c d", d=DPC))

            ident = consts.tile([P, P], F16)
            make_identity(nc, ident)
            ones_row = consts.tile([1, NFREE], F16)
            nc.vector.memset(ones_row, 1.0)

            bq_sb = wpool.tile([1, DPC], F16)
            nc.scalar.dma_start(bq_sb, bqs.ap()[None, :])
            bk_sb = wpool.tile([1, DPC], F16)
            nc.scalar.dma_start(bk_sb, bkp.ap()[None, :])
            bv_sb = wpool.tile([1, DPC], F16)
            nc.scalar.dma_start(bv_sb, bvp.ap()[None, :])

            # ---------- residents (per batch) ----------
            qt_sb, kt_sb, v_sb = [], [], []
            for b in range(B):
                qt_sb.append(resid.tile([P, S], F16, name=f"qt{b}"))
                kt_sb.append(resid.tile([P, S], F16, name=f"kt{b}"))
                vv = resid.tile([P, KC, HPC, HD + 1], F16, name=f"v{b}")
                nc.vector.memset(vv[:, :, :, HD : HD + 1], 1.0)
                v_sb.append(vv)

            # ---------- input DMAs --------------------------------------
            # Half-tiles [128, 4 e-chunks, 512 rows] (4KB/partition
            # contiguous), spread over three rings so the prologue's loads
            # pipeline: e(b0,r0..r2)+e(b1,*) on sync, q+weights on gpsimd,
            # biases+e(b0,r3) on scalar. Allocation order interleaves e/q so
            # the 16 pool slots hold exactly batch 0; batch 1 reuses slots.
            HEC2 = EC // 2
            src = {}

            def load_half(eng, key, dram, b, r, h):
                blk = (b * RC_B + r) * P
                t = srcp.tile(
                    [P, HEC2, NFREE], F16, tag="src", name=f"{key}{b}{r}{h}"
                )
                eng.dma_start(
                    t,
                    dram.ap()[
                        blk : blk + P,
                        h * HEC2 * NFREE : (h + 1) * HEC2 * NFREE,
                    ].rearrange("p (c n) -> p c n", n=NFREE),
                )
                src[(key, b, r, h)] = t

            for h in range(2):
                load_half(nc.sync, "e", eT, 0, 0, h)
            for h in range(2):
                load_half(nc.gpsimd, "q", qT, 0, 0, h)
            for h in range(2):
                load_half(nc.sync, "e", eT, 0, 1, h)
            for h in range(2):
                load_half(nc.gpsimd, "q", qT, 0, 1, h)
            for h in range(2):
                load_half(nc.sync, "e", eT, 0, 2, h)
            for h in range(2):
                load_half(nc.scalar, "e", eT, 0, 3, h)
            for r in (2, 3):
                for h in range(2):
                    load_half(nc.gpsimd, "q", qT, 0, r, h)
            for r in range(RC_B):
                for h in range(2):
                    load_half(nc.sync, "e", eT, 1, r, h)
                for h in range(2):
                    load_half(nc.gpsimd, "q", qT, 1, r, h)

            # ---------- projection half-pieces (~850ns of PE each) ------
            HEC = EC // 2
            qk_acc = {}

            def qk_half(key, b, r, half, w_t, b_t, dst):
                """Half of a Q/K projection chunk: 4 contraction matmuls;
                second half adds bias and copies PSUM -> resident tile."""
                xs = src[(key, b, r, half)]
                if half == 0:
                    qk_acc[(key, b, r)] = pproj.tile(
                        [P, NFREE], F32, tag="pj", name=f"pj_{key}{b}{r}"
                    )
                pp = qk_acc[(key, b, r)]
                for c in range(half * HEC, half * HEC + HEC):
                    nc.tensor.matmul(
                        pp,
                        lhsT=w_t[:, c],
                        rhs=xs[:, c - half * HEC],
                        start=(c == 0),
                        stop=(not with_bias and c == EC - 1),
                    )
                if half == 1:
                    if with_bias:
                        nc.tensor.matmul(
                            pp, lhsT=b_t, rhs=ones_row, start=False, stop=True
                        )
                    col0 = r * NFREE
                    nc.vector.tensor_copy(dst[:, col0 : col0 + NFREE], pp)

            def v_half(b, r, pair):
                """Two 128-row V sub-chunks (kc = 4r+2*pair, +1)."""
                for sub in (2 * pair, 2 * pair + 1):
                    kc = r * (NFREE // P) + sub
                    # ptr pool (not pproj): pproj buf may hold an open Q/K
                    # accumulation across interleaved filler pieces.
                    pv = ptr.tile([P, HPC, HD], F32, tag="tp")
                    for c in range(EC):
                        xs = src[("e", b, r, c // HEC2)]
                        nc.tensor.matmul(
                            pv,
                            lhsT=xs[:, c % HEC2, sub * P : (sub + 1) * P],
                            rhs=wv_sb[:, c],
                            start=(c == 0),
                            stop=(not with_bias and c == EC - 1),
                        )
                    if with_bias:
                        nc.tensor.matmul(
                            pv,
                            lhsT=ones_row[:, :P],
                            rhs=bv_sb,
                            start=False,
                            stop=True,
                        )
                    nc.vector.tensor_copy(v_sb[b][:, kc, :, 0:HD], pv)

            def piece(kind, b, r, i):
                if kind == "K":
                    return lambda: qk_half("e", b, r, i, wk_sb, bk_sb, kt_sb[b])
                if kind == "Q":
                    return lambda: qk_half("q", b, r, i, wq_sb, bq_sb, qt_sb[b])
                return lambda: v_half(b, r, i)

            # ---------- filler queue (proj pieces woven into attention) --
            # Slots are attention groups, numbered globally 0..127.
            SPI = GRP                       # slots per iter
            ipb = QC * HPC                  # iters per batch = 8

            def slot_of(b, it, g):
                return (b * ipb + it) * SPI + g

            fillers = []  # (need_by_slot, piece_fn) sorted by emission
            for b in range(B):
                base = b * ipb * SPI
                nxt = (b + 1) * ipb * SPI
                # V(b, r1..r3): piece (r, pair) feeds PV group 2r+pair of this
                # batch's first iter (slot base+2r+pair); pop one slot earlier.
                for r in range(1, RC_B):
                    for pr_ in range(2):
                        fillers.append(
                            (base + 2 * r + pr_ - 1, piece("V", b, r, pr_))
                        )
                # Q(b, r): feeds iters with qc >= r (first: it = r*HPC);
                # r0 comes from the prologue (b0) / next-batch fillers (b1).
                for r in (1, 2, 3):
                    need = base + (r * HPC) * SPI - 1
                    fillers.append((need - 1, piece("Q", b, r, 0)))
                    fillers.append((need, piece("Q", b, r, 1)))
                # K(b, r): QK is issued 2 groups ahead, so group 2r's QK
                # (reading kt cols of chunk r) is emitted at slot 2r-2.
                if b == 0:
                    fillers.append((0, piece("K", b, 2, 0)))
                    fillers.append((1, piece("K", b, 2, 1)))
                    fillers.append((2, piece("K", b, 3, 0)))
                    fillers.append((3, piece("K", b, 3, 1)))
                if b + 1 < B:
                    # next batch prologue-equivalent (staggered 1/slot):
                    # K r0..r3, then V r0, Q r0 just before the boundary.
                    for r in range(RC_B):
                        fillers.append((nxt - 11 + 2 * r, piece("K", b + 1, r, 0)))
                        fillers.append((nxt - 10 + 2 * r, piece("K", b + 1, r, 1)))
                    fillers.append((nxt - 3, piece("V", b + 1, 0, 0)))
                    fillers.append((nxt - 2, piece("V", b + 1, 0, 1)))
                    fillers.append((nxt - 2, piece("Q", b + 1, 0, 0)))
                    fillers.append((nxt - 1, piece("Q", b + 1, 0, 1)))
            fillers.sort(key=lambda x: x[0])

            AHEAD = 40  # eligibility window (slots) for early emission

            def pop_fillers(slot, force_only=False):
                # forced (dependency) pops: unlimited; early pops: <=1/slot
                while fillers and fillers[0][0] <= slot:
                    fillers.pop(0)[1]()
                if not force_only and fillers and fillers[0][0] <= slot + AHEAD:
                    fillers.pop(0)[1]()

            # ---------- attention ----------
            drains = []   # deferred per-iter drain pieces (run next iter)
            ot_cur = {}   # (b, qc) -> 4 output tiles [128 rows, 128 dpc]

            def emit_qk(b, h, qc, g):
                d0 = h * HD
                col0 = qc * NFREE
                sp = psq.tile([P, KGRP * NFREE], F32, tag="sps")
                for j in range(KGRP):
                    kc = g * KGRP + j
                    nc.tensor.matmul(
                        sp[:, j * NFREE : (j + 1) * NFREE],
                        lhsT=kt_sb[b][d0 : d0 + HD, kc * P : (kc + 1) * P],
                        rhs=qt_sb[b][d0 : d0 + HD, col0 : col0 + NFREE],
                        start=True,
                        stop=True,
                    )
                pr = prp.tile([P, KGRP * NFREE], F16, tag="pr")
                nc.scalar.activation(pr, sp, AF.Exp)
                return pr

            def make_drain(b, h, qc, ctx_ps, ot_tiles):
                """Drain pieces for one finished iter: ctx copy (DVE), then
                per-128-row-block transpose + normalize (+ store at h=1)."""
                ctxT = ctp.tile([HD + 1, NFREE], F16, tag="ctxT")
                d0 = h * HD

                def dcopy():
                    nc.vector.tensor_copy(ctxT, ctx_ps)

                def dblk(t):
                    def run():
                        tp = ptr.tile([P, HD + 1], F16, tag="tp")
                        nc.tensor.transpose(
                            tp,
                            ctxT[:, t * P : (t + 1) * P],
                            ident[: HD + 1, : HD + 1],
                        )
                        rcp = misc.tile([P, 1], F32, tag="rcp")
                        nc.vector.reciprocal(rcp, tp[:, HD : HD + 1])
                        nc.vector.tensor_mul(
                            ot_tiles[t][:, d0 : d0 + HD],
                            tp[:, 0:HD],
                            rcp.broadcast_to([P, HD]),
                        )
                        if h == HPC - 1:
                            row0 = b * S + qc * NFREE + t * P
                            nc.sync.dma_start(
                                out.ap()[row0 : row0 + P, :], ot_tiles[t]
                            )

                    return run

                return [dcopy] + [dblk(t) for t in range(NFREE // P)]

            def attn_iter(b, it):
                qc, h = it // HPC, it % HPC
                if h == 0:
                    ot_cur[(b, qc)] = [
                        otp.tile([P, DPC], F32, tag="ot", name=f"ot{b}_{qc}_{t}")
                        for t in range(NFREE // P)
                    ]
                ot_tiles = ot_cur[(b, qc)]
                ctx_ps = pctx.tile([HD + 1, NFREE], F32, tag="ctx")
                prs = {}
                prs[0] = emit_qk(b, h, qc, 0)
                prs[1] = emit_qk(b, h, qc, 1)
                for g in range(GRP):
                    slot = slot_of(b, it, g)
                    # PV of group g
                    for j in range(KGRP):
                        kc = g * KGRP + j
                        nc.tensor.matmul(
                            ctx_ps,
                            lhsT=v_sb[b][:, kc, h, :],
                            rhs=prs[g][:, j * NFREE : (j + 1) * NFREE],
                            start=(kc == 0),
                            stop=(kc == KC - 1),
                        )
                    if g + 2 < GRP:
                        prs[g + 2] = emit_qk(b, h, qc, g + 2)
                    if drains:
                        drains.pop(0)()
                    pop_fillers(slot, force_only=(slot < SPI))
                drains.extend(make_drain(b, h, qc, ctx_ps, ot_tiles))

            # ---------- prologue: minimal proj before attention ----------
            # K r0..r1 (feeds QK groups 0..3; r2/r3 are slot-0/2 fillers),
            # V r0 (feeds PV groups 0..1), Q r0 (feeds qc0 iters).
            for r in (0, 1):
                qk_half("e", 0, r, 0, wk_sb, bk_sb, kt_sb[0])
                qk_half("e", 0, r, 1, wk_sb, bk_sb, kt_sb[0])
            v_half(0, 0, 0)
            v_half(0, 0, 1)
            qk_half("q", 0, 0, 0, wq_sb, bq_sb, qt_sb[0])
            qk_half("q", 0, 0, 1, wq_sb, bq_sb, qt_sb[0])

            # ---------- main loop ----------
            for b in range(B):
                for it in range(ipb):
                    attn_iter(b, it)
            while drains:
                drains.pop(0)()
            while fillers:
                fillers.pop(0)[1]()

    nc.finalize()
    return nc


def _block_src(x):
    """[B, S, E] fp32 -> [B*RC*P, EC*NFREE] fp16 with tile (b, r) holding
    [p, c*NFREE + n] = x[b, r*NFREE + n, c*P + p] (contiguous per partition)."""
    b5 = x.reshape(B, RC_B, NFREE, EC, P).transpose(0, 1, 4, 3, 2)
    return np.ascontiguousarray(b5).reshape(B * RC_B * P, EC * NFREE).astype(np.float16)


def _block_w(w):
    """[DPC, E] fp32 -> [P, EC*DPC] fp16: [p, c*DPC + d] = w[d, c*P + p]."""
    b3 = w.T.reshape(EC, P, DPC).transpose(1, 0, 2)
    return np.ascontiguousarray(b3).reshape(P, EC * DPC).astype(np.float16)


def _get_nc(with_bias: bool = True) -> bass.Bass:
    if with_bias not in _CACHED_NC:
        _CACHED_NC[with_bias] = _build_nc(with_bias)
    return _CACHED_NC[with_bias]


def kernel(embed, q, Wk, bk, Wq, bq, Wv, bv, trace=False):
    global LAST_RESULTS
    embed = np.asarray(embed, dtype=np.float32)
    q = np.asarray(q, dtype=np.float32)
    Wk = np.asarray(Wk, dtype=np.float32)
    Wq = np.asarray(Wq, dtype=np.float32)
    Wv = np.asarray(Wv, dtype=np.float32)
    bk = np.asarray(bk, dtype=np.float32)
    bq = np.asarray(bq, dtype=np.float32)
    bv = np.asarray(bv, dtype=np.float32)

    qT = _block_src(q)
    eT = _block_src(embed)

    in_maps = []
    for c in range(NCORES):
        sl = slice(c * DPC, (c + 1) * DPC)
        in_maps.append(
            {
                "qT": qT,
                "eT": eT,
                # scores scale folded into Wq/bq (exact: *2^-3)
                "WqT": _block_w(Wq[sl] * SCALE),
                "WkT": _block_w(Wk[sl]),
                "WvT": _block_w(Wv[sl]),
                "bqs": (bq[sl] * SCALE).astype(np.float16),
                "bkp": bk[sl].astype(np.float16),
                "bvp": bv[sl].astype(np.float16),
            }
        )

    with_bias = bool(bq.any() or bk.any() or bv.any())
    nc = _get_nc(with_bias)
    res = run_bass_kernel_spmd(nc, in_maps, list(range(NCORES)), trace=trace)
    LAST_RESULTS = res

    full = np.empty((ROWS, E), dtype=np.float32)
    for c in range(NCORES):
        full[:, c * DPC : (c + 1) * DPC] = res.results[c]["out"]
    return full.reshape(B, S, E)
